# revision 16
# baseline (speedup 1.0000x reference)
"""CIKA conv block on 8 Trainium2 NeuronCores.

Sharding: pure data parallel. 8 shards = (batch n, H half). Each core gets
zero-padded, W-strip-interleaved bf16 slices plus replicated weights, and
computes its (32, 128, 256) slice of both outputs (low, up).

On-chip layout: [128 partitions = 4 W-strips x 32 channels].

v2 structure:
 - Depthwise 5x5 convs contract FOUR taps per matmul: the host ships
   row-shift replica tensors [128 = 4 shifts x 32 ch, rows, 68] per strip;
   each (row-offset d, col J) group is one K=128 matmul whose lhsT is
   block-diagonal over the shift axis.  The four strips run as four
   concurrent PE column tiles (tile_position=(0, 32s)); each column slice
   carries its own start/stop (has_written clearing is per tile).
 - The dynamic-conv loop is DVE-bound (scalar_tensor_tensor runs in 1x
   mode with a PSUM operand).  To keep the PE busy under it, emission is
   software-pipelined: the W_low consumer lags one tap behind the
   sel/STT producers, and the NEXT tile's whole PE/ACT chain (KCA, UP
   branch, KSA) is interleaved into the dyn steps so the FIFO engine
   queues stay fed.
 - low/up biases fold into PSUM as K=1 matmuls (bias row x ones); the
   evacuations are ScalarE copies; outputs leave as bf16 (host upcasts).
"""

import os
from contextlib import ExitStack

import numpy as np

import concourse.bacc as bacc
import concourse.bass as bass
import concourse.mybir as mybir
import concourse.tile as tile
from concourse.bass_utils import run_bass_kernel_spmd

F32 = mybir.dt.float32
BF16 = mybir.dt.bfloat16
AF = mybir.ActivationFunctionType
ALU = mybir.AluOpType

KK = 5          # kernel size
CH = 32         # channels
NB, H, W = 4, 256, 256
N_CORES = 8
HSH = H // 2    # rows per core (one batch-half per core)
ROWS_T = 32     # output rows per on-chip tile
NT = HSH // ROWS_T
SW = 64         # strip width (W / 4)
RH = 130        # replica-tensor rows (max tile slice r0=96 .. r0+34)
TAPS = [(i, j) for i in range(KK) for j in range(KK)]
# dw tap groups: (row offset d, col J); d=0 covers taps i=0..3 via the four
# shift blocks, d=1 uses only block g=3 for the i=4 row.
DW_GROUPS = [(0, j) for j in range(KK)] + [(1, j) for j in range(KK)]

LAST_EXEC_NS = None


def _emit(ctx: ExitStack, tc: tile.TileContext, io):
    nc = tc.nc
    (lower_d, lowrep_d, uprep_d, wdwg_d, sel_d, wm1_d, wm2_d, wk1_d, wk2_d,
     wlow_d, wup_d, wdyn_d, bias_d, brow_d, ones_d, low_od, up_od) = io

    wpool = ctx.enter_context(tc.tile_pool(name="wts", bufs=1))
    inp = ctx.enter_context(tc.tile_pool(name="inp", bufs=2))
    work = ctx.enter_context(tc.tile_pool(name="work", bufs=2))
    outp = ctx.enter_context(tc.tile_pool(name="outp", bufs=2))
    ps_dw = ctx.enter_context(tc.tile_pool(name="psdw", bufs=2, space="PSUM"))
    ps_pw = ctx.enter_context(tc.tile_pool(name="pspw", bufs=2, space="PSUM"))
    ps_rep = ctx.enter_context(tc.tile_pool(name="psrep", bufs=3,
                                            space="PSUM"))
    ps_low = ctx.enter_context(tc.tile_pool(name="pslow", bufs=1,
                                            space="PSUM"))

    # ---- load weights once (all bf16 except biases) ----
    w_dwg = wpool.tile([128, 30, 32], BF16)
    nc.sync.dma_start(w_dwg[:], wdwg_d[:])
    sel = wpool.tile([128, 25, 128], BF16)
    nc.sync.dma_start(sel[:], sel_d[:])
    w_m1 = wpool.tile([128, 32], BF16)
    nc.sync.dma_start(w_m1[:], wm1_d[:])
    w_m2 = wpool.tile([32, 128], BF16)
    nc.sync.dma_start(w_m2[:], wm2_d[:])
    w_k1 = wpool.tile([128, 100], BF16)
    nc.sync.dma_start(w_k1[:], wk1_d[:])
    w_k2 = wpool.tile([100, 64], BF16)
    nc.sync.dma_start(w_k2[:], wk2_d[:])
    w_low = wpool.tile([128, 128], BF16)
    nc.sync.dma_start(w_low[:], wlow_d[:])
    w_up = wpool.tile([128, 128], BF16)
    nc.sync.dma_start(w_up[:], wup_d[:])
    wdyn = wpool.tile([128, 25], F32)
    nc.sync.dma_start(wdyn[:], wdyn_d[:])
    # bias columns [128, 9]: 0 b_kca_dw, 1 b_ksa_dw, 2 b_up_dw, 3 b_m1(32),
    # 4 b_m2, 5 b_k1(100), 6 b_k2(64-slot), 7/8 spare
    bias = wpool.tile([128, 9], F32)
    nc.sync.dma_start(bias[:], bias_d[:])
    # bias rows for the K=1 fold matmuls: 0 b_low, 1 b_up_pw
    brow = wpool.tile([1, 2, 128], BF16)
    nc.sync.dma_start(brow[:], brow_d[:])
    ones_t = wpool.tile([1, 8, SW], BF16)
    nc.sync.dma_start(ones_t[:], ones_d[:])

    def bcol(idx, p=128):
        return bias[0:p, idx:idx + 1]

    # PE can encode only one sync wait per matmul (LDWEIGHTS struct limit).
    # Warm-up matmuls make PE observe every weight-DMA queue once, so real
    # matmuls transitively need no weight waits — just their rhs producer.
    sc = ps_pw.tile([1, 1], F32, tag="pspw")
    for wap in (w_dwg[0:1, 0, 0:1], sel[0:1, 0, 0:1], w_m1[0:1, 0:1],
                w_m2[0:1, 0:1], w_k1[0:1, 0:1], w_k2[0:1, 0:1],
                w_low[0:1, 0:1], w_up[0:1, 0:1]):
        nc.tensor.matmul(sc[:], wap, wap, start=True, stop=True)
    nc.tensor.matmul(sc[:], brow[0:1, 0, 0:1], ones_t[0:1, 0, 0:1],
                     start=True, stop=True)

    def dma_tile(it):
        r0 = it * ROWS_T
        st = {}
        st["low_t"] = inp.tile([128, ROWS_T + 4, SW + 4], BF16, tag="low_in", name="low_t")
        st["lrep"] = inp.tile([128, 4, 34, SW + 4], BF16, tag="lrep", name="lrep")
        st["urep"] = inp.tile([128, 4, 34, SW + 4], BF16, tag="urep", name="urep")
        nc.sync.dma_start(st["low_t"][:], lower_d[:, r0:r0 + ROWS_T + 4, :])
        nc.sync.dma_start(st["lrep"][:], lowrep_d[:, :, r0:r0 + 34, :])
        nc.sync.dma_start(st["urep"][:], uprep_d[:, :, r0:r0 + 34, :])
        return st

    def dw_burst(ps, reps, cv, q, gi):
        # one (d, J) group: 4 concurrent column-tile matmuls (one per strip)
        d, J = DW_GROUPS[gi]
        for s in range(4):
            nc.tensor.matmul(
                ps[32 * s:32 * (s + 1), :, :],
                w_dwg[:, cv * 10 + gi, :],
                reps[:, s, q * 8 + d:q * 8 + d + 8, J:J + SW],
                start=(gi == 0), stop=(gi == len(DW_GROUPS) - 1),
                tile_position=(0, 32 * s), skip_group_check=True)

    def chain_units(it, st):
        """PE/ACT work of tile `it` that does not depend on its dyn loop:
        KCA chain, UP branch (dw+gate+pw), KSA chain.  Yields thunks."""
        st["t_kca"] = work.tile([128, ROWS_T, SW], BF16, tag="t_kca", name="t_kca")
        st["m1o"] = work.tile([32, ROWS_T, SW], BF16, tag="m1o", name="m1o")
        st["kca"] = work.tile([128, ROWS_T, SW], BF16, tag="kca", name="kca")
        st["t_ksa"] = work.tile([128, ROWS_T, SW], BF16, tag="t_ksa", name="t_ksa")
        st["k1o"] = work.tile([100, 2, ROWS_T, SW], BF16, tag="k1o", name="k1o")
        st["ksa"] = work.tile([128, ROWS_T, SW], BF16, tag="ksa", name="ksa")
        st["gated"] = work.tile([128, ROWS_T, SW], BF16, tag="gated", name="gated")
        st["up_o"] = outp.tile([128, ROWS_T, SW], BF16, tag="up_o", name="up_o")
        r0 = it * ROWS_T

        def dw_chunk(ps, reps, cv, q, g0, g1):
            # atomic run of tiled-mode bursts (PE tiling-mode switches
            # drain the array — batch them)
            for gi in range(g0, g1):
                dw_burst(ps, reps, cv, q, gi)

        # --- KCA: dw5(lower) relu -> 1x1 (32->8) relu -> 1x1 (8->32) sig ---
        for q in range(4):
            ps = ps_dw.tile([128, 8, SW], F32, tag="psdw")
            yield lambda ps=ps, q=q: dw_chunk(ps, st["lrep"], 0, q, 0, 5)
            yield lambda ps=ps, q=q: dw_chunk(ps, st["lrep"], 0, q, 5, 10)
            yield lambda ps=ps, q=q: nc.scalar.activation(
                st["t_kca"][:, q * 8:(q + 1) * 8, :], ps[:],
                AF.Relu, bias=bcol(0))

        def mm_act(out_sb, w, in_sb, q, func, bias_ap, p=128):
            ps = ps_pw.tile([p, 8, SW], F32, tag="pspw")
            nc.tensor.matmul(ps[:], w, in_sb, start=True, stop=True)
            nc.scalar.activation(out_sb, ps[:], func, bias=bias_ap)

        for q in range(4):
            yield lambda q=q: mm_act(
                st["m1o"][:, q * 8:(q + 1) * 8, :], w_m1[:],
                st["t_kca"][:, q * 8:(q + 1) * 8, :], q, AF.Relu,
                bcol(3, 32), 32)
        for q in range(4):
            yield lambda q=q: mm_act(
                st["kca"][:, q * 8:(q + 1) * 8, :], w_m2[:],
                st["m1o"][:, q * 8:(q + 1) * 8, :], q, AF.Sigmoid, bcol(4))

        # --- UP branch: dw5(upper)+b gated by kca (DVE), then 1x1 + bias ---
        for q in range(4):
            ps = ps_dw.tile([128, 8, SW], F32, tag="psdw")
            yield lambda ps=ps, q=q: dw_chunk(ps, st["urep"], 2, q, 0, 5)
            yield lambda ps=ps, q=q: dw_chunk(ps, st["urep"], 2, q, 5, 10)
            yield lambda ps=ps, q=q: nc.vector.scalar_tensor_tensor(
                st["gated"][:, q * 8:(q + 1) * 8, :], ps[:], bcol(2),
                st["kca"][:, q * 8:(q + 1) * 8, :], ALU.add, ALU.mult)

        def up_pw(q):
            ps = ps_pw.tile([128, 8, SW], F32, tag="pspw")
            nc.tensor.matmul(ps[:], w_up[:],
                             st["gated"][:, q * 8:(q + 1) * 8, :],
                             start=True, stop=False)
            nc.tensor.matmul(ps[:], brow[0:1, 1, :], ones_t[0:1],
                             start=False, stop=True)
            nc.scalar.activation(st["up_o"][:, q * 8:(q + 1) * 8, :], ps[:],
                                 AF.Copy)

        for q in range(4):
            yield lambda q=q: up_pw(q)
        yield lambda: nc.sync.dma_start(up_od[:, r0:r0 + ROWS_T, :],
                                        st["up_o"][:])

        # --- KSA: dw5(upper) relu -> 1x1 (64->100) relu -> (100->64) sig ---
        for q in range(4):
            ps = ps_dw.tile([128, 8, SW], F32, tag="psdw")
            yield lambda ps=ps, q=q: dw_chunk(ps, st["urep"], 1, q, 0, 5)
            yield lambda ps=ps, q=q: dw_chunk(ps, st["urep"], 1, q, 5, 10)
            yield lambda ps=ps, q=q: nc.scalar.activation(
                st["t_ksa"][:, q * 8:(q + 1) * 8, :], ps[:],
                AF.Relu, bias=bcol(1))
        for g in range(2):
            for q in range(4):
                def k1_unit(g=g, q=q):
                    ps = ps_pw.tile([100, 8, SW], F32, tag="pspw")
                    nc.tensor.matmul(
                        ps[:], w_k1[g * 64:(g + 1) * 64, :],
                        st["t_ksa"][g * 64:(g + 1) * 64,
                                    q * 8:(q + 1) * 8, :],
                        start=True, stop=True)
                    nc.scalar.activation(st["k1o"][:, g, q * 8:(q + 1) * 8,
                                                   :], ps[:],
                                         AF.Relu, bias=bcol(5, 100))
                yield k1_unit
        for g in range(2):
            for q in range(4):
                def k2_unit(g=g, q=q):
                    ps = ps_pw.tile([64, 8, SW], F32, tag="pspw")
                    nc.tensor.matmul(ps[:], w_k2[:],
                                     st["k1o"][:, g, q * 8:(q + 1) * 8, :],
                                     start=True, stop=True)
                    nc.scalar.activation(
                        st["ksa"][64 * g:64 * (g + 1), q * 8:(q + 1) * 8, :],
                        ps[:], AF.Sigmoid, bias=bcol(6, 64))
                yield k2_unit

    def emit_dyn(it, st, fillers):
        """Dyn loop of tile `it`, one q-chunk at a time.  All matmuls here
        are full 128x128 mode (no PE tiling-mode switches).  The sel/STT
        producers lead the W_low consumer by two taps; `fillers` (chain
        units of the next tile) drain between steps."""
        r0 = it * ROWS_T
        low_t, ksa = st["low_t"], st["ksa"]
        st["low_o"] = outp.tile([128, ROWS_T, SW], BF16, tag="low_o",
                                name="low_o")
        n_fill = len(fillers)
        fi = [0]

        def fill(upto):
            while fi[0] < upto and fillers:
                fillers.pop(0)()
                fi[0] += 1

        nstt = 0
        seq = [(q, t) for q in range(4) for t in range(len(TAPS))]
        lps = {}
        first = {}

        def emit_wlow(u):
            q, t = seq[u]
            if q not in lps:
                lps[q] = ps_low.tile([128, 8, SW], F32, tag="pslow",
                                     name="lp")
            nc.tensor.matmul(lps[q][:], w_low[:], mts.pop(u)[:],
                             start=(t == 0), stop=False)
            if t == len(TAPS) - 1:
                nc.tensor.matmul(lps[q][:], brow[0:1, 0, :], ones_t[0:1],
                                 start=False, stop=True)
                nc.scalar.activation(st["low_o"][:, q * 8:(q + 1) * 8, :],
                                     lps.pop(q)[:], AF.Copy)

        mts = {}
        for u in range(len(seq) + 2):
            if u < len(seq):
                q, t = seq[u]
                i, j = TAPS[t]
                rep = ps_rep.tile([128, 8, SW], F32, tag="rep", name="rep")
                nc.tensor.matmul(rep[:], sel[:, t, :],
                                 ksa[:, q * 8:(q + 1) * 8, :],
                                 start=True, stop=True)
                mt = work.tile([128, 8, SW], BF16, tag=f"mt{nstt % 4}",
                               name="mt")
                nc.vector.scalar_tensor_tensor(
                    mt[:], low_t[:, q * 8 + i:q * 8 + i + 8, j:j + SW],
                    wdyn[:, t:t + 1], rep[:], ALU.mult, ALU.mult)
                mts[u] = mt
                nstt += 1
                fill(n_fill * nstt // 100)
            if u >= 2:
                emit_wlow(u - 2)
        nc.sync.dma_start(low_od[:, r0:r0 + ROWS_T, :], st["low_o"][:])

    # ---- main software pipeline over tiles ----
    states = [None] * NT
    states[0] = dma_tile(0)
    prologue = list(chain_units(0, states[0]))
    for u in prologue:
        u()
    for it in range(NT):
        if it + 1 < NT:
            states[it + 1] = dma_tile(it + 1)
            fillers = list(chain_units(it + 1, states[it + 1]))
        else:
            fillers = []
        emit_dyn(it, states[it], fillers)
        for u in fillers:       # drain any leftovers
            u()


_NC_CACHE = {}


def _build_nc():
    if "nc" in _NC_CACHE:
        return _NC_CACHE["nc"]
    nc = bacc.Bacc("TRN2", target_bir_lowering=False)
    lower_d = nc.dram_tensor("lower_sh", (128, HSH + 4, SW + 4), BF16,
                             kind="ExternalInput")
    lowrep_d = nc.dram_tensor("lowrep", (128, 4, RH, SW + 4), BF16,
                              kind="ExternalInput")
    uprep_d = nc.dram_tensor("uprep", (128, 4, RH, SW + 4), BF16,
                             kind="ExternalInput")
    wdwg_d = nc.dram_tensor("w_dwg", (128, 30, 32), BF16,
                            kind="ExternalInput")
    sel_d = nc.dram_tensor("sel", (128, 25, 128), BF16, kind="ExternalInput")
    wm1_d = nc.dram_tensor("w_m1", (128, 32), BF16, kind="ExternalInput")
    wm2_d = nc.dram_tensor("w_m2", (32, 128), BF16, kind="ExternalInput")
    wk1_d = nc.dram_tensor("w_k1", (128, 100), BF16, kind="ExternalInput")
    wk2_d = nc.dram_tensor("w_k2", (100, 64), BF16, kind="ExternalInput")
    wlow_d = nc.dram_tensor("w_low", (128, 128), BF16, kind="ExternalInput")
    wup_d = nc.dram_tensor("w_up", (128, 128), BF16, kind="ExternalInput")
    wdyn_d = nc.dram_tensor("w_dyn", (128, 25), F32, kind="ExternalInput")
    bias_d = nc.dram_tensor("biases", (128, 9), F32, kind="ExternalInput")
    brow_d = nc.dram_tensor("brow", (1, 2, 128), BF16, kind="ExternalInput")
    ones_d = nc.dram_tensor("ones", (1, 8, SW), BF16, kind="ExternalInput")
    low_od = nc.dram_tensor("low_out", (128, HSH, SW), BF16,
                            kind="ExternalOutput")
    up_od = nc.dram_tensor("up_out", (128, HSH, SW), BF16,
                           kind="ExternalOutput")
    io = (lower_d, lowrep_d, uprep_d, wdwg_d, sel_d, wm1_d, wm2_d, wk1_d,
          wk2_d, wlow_d, wup_d, wdyn_d, bias_d, brow_d, ones_d, low_od,
          up_od)
    with tile.TileContext(nc) as tc:
        with ExitStack() as ctx:
            _emit(ctx, tc, io)
    nc.compile()
    _NC_CACHE["nc"] = nc
    return nc


def _prep_weights(kca_dw_w, kca_dw_b, kca_m1_w, kca_m1_b, kca_m2_w, kca_m2_b,
                  ksa_dw_w, ksa_dw_b, ksa_m1_w, ksa_m1_b, ksa_m2_w, ksa_m2_b,
                  low_dyn_w, low_dyn_b, low_pw_w, low_pw_b,
                  up_dw_w, up_dw_b, up_pw_w, up_pw_b):
    f = np.float32
    import ml_dtypes
    bf = ml_dtypes.bfloat16
    ar = np.arange(32)
    # grouped dw weights: per conv, 10 groups (d=0: taps i=0..3 via shift
    # blocks; d=1: only block g=3 carries tap i=4)
    w_dwg = np.zeros((128, 30, 32), f)
    for cv, wt in enumerate([kca_dw_w, ksa_dw_w, up_dw_w]):
        w5 = np.asarray(wt, f).reshape(CH, KK, KK)
        for gi, (d, J) in enumerate(DW_GROUPS):
            for g in range(4):
                i = d + g
                if d == 1 and g != 3:
                    continue
                w_dwg[g * 32 + ar, cv * 10 + gi, ar] = w5[:, i, J]
    sel = np.zeros((128, 25, 128), f)
    for s in range(4):
        for t in range(25):
            sel[s * 32 + t, t, s * 32:(s + 1) * 32] = 1.0
    i4, i2 = np.eye(4, dtype=f), np.eye(2, dtype=f)
    w_m1 = np.kron(i4, np.asarray(kca_m1_w, f).T)        # (128, 32)
    w_m2 = np.kron(i4, np.asarray(kca_m2_w, f).T)        # (32, 128)
    w_k1 = np.kron(i2, np.asarray(ksa_m1_w, f).T)        # (64, 100)
    w_k1 = np.vstack([w_k1, w_k1])                       # (128, 100) dup
    w_k2 = np.zeros((100, 64), f)                        # padded to 32-slots
    w2t = np.asarray(ksa_m2_w, f).T                      # (50, 25)
    for sl in range(2):
        w_k2[sl * 50:(sl + 1) * 50, sl * 32:sl * 32 + 25] = w2t
    w_low = np.kron(i4, np.asarray(low_pw_w, f).T)       # (128, 128)
    w_up = np.kron(i4, np.asarray(up_pw_w, f).T)         # (128, 128)
    w_dyn = np.tile(np.asarray(low_dyn_w, f).reshape(CH, 25), (4, 1))
    bias = np.zeros((128, 9), f)
    bias[:, 0] = np.tile(np.asarray(kca_dw_b, f), 4)
    bias[:, 1] = np.tile(np.asarray(ksa_dw_b, f), 4)
    bias[:, 2] = np.tile(np.asarray(up_dw_b, f), 4)
    bias[:32, 3] = np.tile(np.asarray(kca_m1_b, f), 4)
    bias[:, 4] = np.tile(np.asarray(kca_m2_b, f), 4)
    bias[:100, 5] = np.tile(np.asarray(ksa_m1_b, f), 2)
    for sl in range(2):
        bias[sl * 32:sl * 32 + 25, 6] = np.asarray(ksa_m2_b, f)
    b_low = np.asarray(low_pw_w, f) @ np.asarray(low_dyn_b, f).reshape(CH) \
        + np.asarray(low_pw_b, f)
    brow = np.zeros((1, 2, 128), f)
    brow[0, 0, :] = np.tile(b_low, 4)
    brow[0, 1, :] = np.tile(np.asarray(up_pw_b, f), 4)
    ones = np.ones((1, 8, SW), f)
    return dict(w_dwg=w_dwg.astype(bf), sel=sel.astype(bf),
                w_m1=w_m1.astype(bf), w_m2=w_m2.astype(bf),
                w_k1=w_k1.astype(bf), w_k2=w_k2.astype(bf),
                w_low=w_low.astype(bf), w_up=w_up.astype(bf),
                w_dyn=w_dyn, biases=bias, brow=brow.astype(bf),
                ones=ones.astype(bf))


def kernel(lower, upper, **wts):
    global LAST_EXEC_NS
    import ml_dtypes
    bf = ml_dtypes.bfloat16
    nc = _build_nc()
    wmap = _prep_weights(**wts)
    lp = np.pad(np.ascontiguousarray(np.asarray(lower, np.float32)),
                ((0, 0), (0, 0), (2, 2), (2, 2))).astype(bf)
    up = np.pad(np.ascontiguousarray(np.asarray(upper, np.float32)),
                ((0, 0), (0, 0), (2, 2), (2, 2))).astype(bf)

    def stripe(x):
        # (32, 132, 260) -> (128 = strip*32+c, 132, 68), strips overlap by 4
        out = np.empty((128, HSH + 4, SW + 4), bf)
        for s in range(4):
            out[s * 32:(s + 1) * 32] = x[:, :, s * SW:s * SW + SW + 4]
        return out

    def replicas(x):
        # (32, 132, 260) -> (128 = shift*32+c, strip, RH, 68)
        xr = np.pad(x, ((0, 0), (0, 1), (0, 0)))     # 133 rows
        out = np.empty((128, 4, RH, SW + 4), bf)
        for g in range(4):
            for s in range(4):
                out[g * 32:(g + 1) * 32, s] = \
                    xr[:, g:g + RH, s * SW:s * SW + SW + 4]
        return out

    in_maps = []
    for k in range(N_CORES):
        n, half = k // 2, k % 2
        m = dict(wmap)
        lsl = lp[n, :, half * HSH:half * HSH + HSH + 4, :]
        usl = up[n, :, half * HSH:half * HSH + HSH + 4, :]
        m["lower_sh"] = stripe(lsl)
        m["lowrep"] = replicas(lsl)
        m["uprep"] = replicas(usl)
        in_maps.append(m)
    trace = os.environ.get("BASS_KERNEL_TRACE", "0") == "1"
    res = run_bass_kernel_spmd(nc, in_maps, core_ids=list(range(N_CORES)),
                               trace=trace)
    LAST_EXEC_NS = res.exec_time_ns
    low = np.empty((NB, CH, H, W), np.float32)
    upo = np.empty((NB, CH, H, W), np.float32)
    for k in range(N_CORES):
        n, half = k // 2, k % 2
        lo = res.results[k]["low_out"].astype(np.float32)
        uo = res.results[k]["up_out"].astype(np.float32)
        for s in range(4):
            low[n, :, half * HSH:(half + 1) * HSH, s * SW:(s + 1) * SW] = \
                lo[s * 32:(s + 1) * 32]
            upo[n, :, half * HSH:(half + 1) * HSH, s * SW:(s + 1) * SW] = \
                uo[s * 32:(s + 1) * 32]
    return low, upo
